# revision 17
# baseline (speedup 1.0000x reference)
"""Trainium2 Bass kernel for nn_BilinearLinformerCapsuleFC.

Strategy: data-parallel over batch (32 -> 4 per core x 8 cores).
Per core, per batch item:
  conv1/convq as block-diagonal grouped-conv matmuls (9 shifted taps,
  PSUM-accumulated), Linformer key projection per capsule, keys/queries
  transposed on the PE, then column-softmax attention computed in
  S^T [keys, queries] layout: row-group-packed K@Q^T matmuls -> one big
  ACT exp with accum_out row-sum (the softmax-over-queries normalizer)
  -> K' = K/Z -> col-group-packed K'^T @ E matmuls accumulating Out^T in
  a single PSUM bank. Output conv + LayerNorm (rsqrt via exp(-0.5 ln))
  with DRAM-roundtrip relayouts for the 16<->49 flat reinterpretations.
"""
import numpy as np
import ml_dtypes

import concourse.bass as bass
import concourse.mybir as mybir
import concourse.tile as tile
from concourse import bacc
from concourse.bass import ds
from concourse.bass_utils import run_bass_kernel_spmd

BF16 = mybir.dt.bfloat16
F32 = mybir.dt.float32
AF = mybir.ActivationFunctionType
ALU = mybir.AluOpType

B, IN_N, IN_D, H, OUT_N, OUT_D, HO, HID = 32, 32, 16, 14, 32, 16, 7, 64
C = IN_N * IN_D            # 512
NB = 4                     # batch items per core
NKEY = IN_N * HID + HO * HO  # 2097
J = OUT_N * HO * HO        # 1568
NT_I = 17                  # i tiles (16x128 + 49)
NT_J = 13                  # j tiles (12x128 + 32)
JPAD = NT_J * 2048         # 26624, padded flat len per batch
EPS = 1e-5
CHUNKS = [(0, 512), (512, 512), (1024, 512), (1536, 32)]  # j chunks
SCALE = IN_D ** -0.5       # 0.25

_PROG = None


def _rows_i(it):
    return 128 if it < 16 else NKEY - 16 * 128  # 49


def _rows_j(jt):
    return 128 if jt < 12 else J - 12 * 128  # 32


def _build():
    nc = bacc.Bacc("TRN2", target_bir_lowering=False, debug=False, num_devices=1)

    cp_img = nc.dram_tensor("cp_img", [NB, C, 16, 16], BF16, kind="ExternalInput")
    qp_img = nc.dram_tensor("qp_img", [NB, C, 9, 9], BF16, kind="ExternalInput")
    w1bd = nc.dram_tensor("w1bd", [9, 4, 128, 128], BF16, kind="ExternalInput")
    wqbd = nc.dram_tensor("wqbd", [9, 4, 128, 128], BF16, kind="ExternalInput")
    ep = nc.dram_tensor("ep", [IN_N, 196, HID], BF16, kind="ExternalInput")
    rel_k = nc.dram_tensor("rel_k", [49, 16], F32, kind="ExternalInput")
    rel_kt = nc.dram_tensor("rel_kt", [16, 49], BF16, kind="ExternalInput")
    gam_d = nc.dram_tensor("gam", [16], F32, kind="ExternalInput")
    bet_d = nc.dram_tensor("bet", [16], F32, kind="ExternalInput")
    ident_d = nc.dram_tensor("ident", [128, 128], BF16, kind="ExternalInput")
    bident_d = nc.dram_tensor("bident", [128, 16], BF16, kind="ExternalInput")

    fq = nc.dram_tensor("fq", [NB, JPAD], BF16)
    fo = nc.dram_tensor("fo", [NB, JPAD], BF16)
    f3 = nc.dram_tensor("f3", [NB, JPAD], F32)
    outy = nc.dram_tensor("outy", [NB, JPAD], F32, kind="ExternalOutput")

    def dmae(i):
        return nc.sync

    with tile.TileContext(nc) as tc:
        from contextlib import ExitStack
        with ExitStack() as ctx:
            consts = ctx.enter_context(tc.tile_pool(name="consts", bufs=1))
            big = ctx.enter_context(tc.tile_pool(name="big", bufs=1))
            perb = ctx.enter_context(tc.tile_pool(name="perb", bufs=2))
            ebuf = ctx.enter_context(tc.tile_pool(name="ebuf", bufs=2))
            small = ctx.enter_context(tc.tile_pool(name="small", bufs=3))
            pp_s = ctx.enter_context(tc.tile_pool(name="pp_s", bufs=1, space="PSUM"))
            pp_o = ctx.enter_context(tc.tile_pool(name="pp_o", bufs=2, space="PSUM"))
            pp_sm = ctx.enter_context(tc.tile_pool(name="pp_sm", bufs=2, space="PSUM"))

            # ---- constants ----
            ident = consts.tile([128, 128], BF16)
            nc.sync.dma_start(out=ident, in_=ident_d[:, :])
            bident = consts.tile([128, 16], BF16)
            nc.sync.dma_start(out=bident, in_=bident_d[:, :])
            w1s = consts.tile([128, 9, 4, 128], BF16)
            nc.sync.dma_start(out=w1s, in_=bass.AP(
                tensor=w1bd, offset=0,
                ap=[[128, 128], [4 * 128 * 128, 9], [128 * 128, 4], [1, 128]]))
            wqs = consts.tile([128, 9, 4, 128], BF16)
            nc.sync.dma_start(out=wqs, in_=bass.AP(
                tensor=wqbd, offset=0,
                ap=[[128, 128], [4 * 128 * 128, 9], [128 * 128, 4], [1, 128]]))
            eps_s = consts.tile([98, 2, IN_N, HID], BF16)
            for hf in range(2):
                nc.sync.dma_start(out=eps_s[:, hf, :, :], in_=bass.AP(
                    tensor=ep, offset=hf * 98 * HID,
                    ap=[[HID, 98], [196 * HID, IN_N], [1, HID]]))
            gam_b = consts.tile([128, NT_J, 16], F32)
            nc.sync.dma_start(out=gam_b, in_=bass.AP(
                tensor=gam_d, offset=0, ap=[[0, 128], [0, NT_J], [1, 16]]))
            bet_b = consts.tile([128, NT_J, 16], F32)
            nc.sync.dma_start(out=bet_b, in_=bass.AP(
                tensor=bet_d, offset=0, ap=[[0, 128], [0, NT_J], [1, 16]]))
            eps_t = consts.tile([128, 1], F32)
            nc.vector.memset(eps_t, EPS)
            zpad = consts.tile([96, 16], F32)
            nc.vector.memset(zpad, 0.0)

            # ---- phase A: inputs come pre-padded from host ----
            x0 = big.tile([128, NB, 4, 16, 16], BF16)
            x0q = big.tile([128, NB, 4, 9, 9], BF16)
            for b in range(NB):
                for blk in range(4):
                    nc.sync.dma_start(out=x0[:, b, blk, :, :],
                                      in_=cp_img[b, blk * 128:(blk + 1) * 128, :, :])
                    nc.sync.dma_start(out=x0q[:, b, blk, :, :],
                                      in_=qp_img[b, blk * 128:(blk + 1) * 128, :, :])

            # ---- conv1 (block-diag, tap-outer for weight reuse) ----
            x1 = big.tile([98, NB, 2, C], BF16)   # X1^T: [s_lo, b, s_half, c]
            for blk in range(4):
                for b in range(NB):
                    pc = pp_sm.tile([128, 196], F32, tag="sm")
                    for tap in range(9):
                        ky, kx = tap // 3, tap % 3
                        nc.tensor.matmul(
                            out=pc[:, :], lhsT=w1s[:, tap, blk, :],
                            rhs=x0[:, b, blk, ky:ky + 14, kx:kx + 14],
                            start=(tap == 0), stop=(tap == 8))
                    x1c = small.tile([128, 196], BF16, tag="x1c")
                    nc.vector.tensor_copy(out=x1c, in_=pc)
                    # transpose [128,196] -> two [98,128] into X1^T
                    for hf in range(2):
                        pt = pp_sm.tile([98, 128], BF16, tag="sm")
                        nc.tensor.transpose(pt, x1c[:, hf * 98:(hf + 1) * 98], ident)
                        nc.vector.tensor_copy(
                            out=x1[:, b, hf, blk * 128:(blk + 1) * 128], in_=pt)

            # ---- k projection (per capsule), K assembly ----
            kt_rep = big.tile([128, NB, 2112], BF16)  # K^T replicas at part 0/32/64/96
            k_sb = big.tile([128, NB, NT_I, 16], F32)
            for b in range(NB):
                nc.sync.dma_start(out=k_sb[0:49, b, 16, :], in_=rel_k[:, :])
            for m in range(2):          # capsule halves (16 each)
                for b in range(NB):
                    pk = pp_sm.tile([128, 8, 16], F32, tag="sm")
                    for q in range(16):
                        n2 = m * 16 + q
                        pslice = pk[64 * (q % 2):64 * (q % 2) + 64, q // 2, :]
                        tp = (0, 64) if (q % 2) else (0, 0)
                        for hf in range(2):
                            nc.tensor.matmul(
                                out=pslice, lhsT=eps_s[:, hf, n2, :],
                                rhs=x1[:, b, hf, n2::32],
                                start=(hf == 0), stop=(hf == 1),
                                tile_position=tp)
                    nc.vector.tensor_copy(out=k_sb[:, b, m * 8:(m + 1) * 8, :], in_=pk)

            # K -> bf16 -> K^T via PE transpose; replicate to row groups
            for b in range(NB):
                kbf = perb.tile([128, 16, 16], BF16, tag="kbf")
                nc.vector.tensor_copy(out=kbf, in_=k_sb[:, b, 0:16, :])
                for grp in range(4):
                    pt = pp_sm.tile([16, 512], BF16, tag="sm")
                    for u in range(4):
                        t = grp * 4 + u
                        nc.tensor.transpose(pt[:, u * 128:(u + 1) * 128],
                                            kbf[:, t, :], ident)
                    nc.scalar.copy(
                        out=kt_rep[0:16, b, grp * 512:(grp + 1) * 512], in_=pt)
                nc.sync.dma_start(out=kt_rep[0:16, b, 2048:2097], in_=rel_kt[:, :])
                for g in range(1, 4):
                    nc.sync.dma_start(out=kt_rep[32 * g:32 * g + 16, b, 0:2097],
                                      in_=kt_rep[0:16, b, 0:2097])

            # ---- convq -> fq roundtrip -> Q^T ----
            qt_rep = big.tile([128, NB, 1568], BF16)
            for blk in range(4):
                for b in range(NB):
                    pc = pp_sm.tile([128, 49], F32, tag="sm")
                    for tap in range(9):
                        ky, kx = tap // 3, tap % 3
                        nc.tensor.matmul(
                            out=pc[:, :], lhsT=wqs[:, tap, blk, :],
                            rhs=x0q[:, b, blk, ky:ky + 7, kx:kx + 7],
                            start=(tap == 0), stop=(tap == 8))
                    xqc = small.tile([128, 49], BF16, tag="xqc")
                    nc.vector.tensor_copy(out=xqc, in_=pc)
                    nc.sync.dma_start(
                        out=fq[b, ds(blk * 6272, 6272)].rearrange("(p s) -> p s", p=128),
                        in_=xqc)
            for b in range(NB):
                # p-major reload: partition p holds flat [208p, 208p+208)
                q_all = perb.tile([128, NT_J, 16], BF16, tag="qall")
                nc.sync.dma_start(
                    out=q_all,
                    in_=fq[b, :].rearrange("(p t e) -> p t e", p=128, e=16))
                for t in range(NT_J):
                    # transpose j-rows {13p + t} -> QT cols strided by 13
                    pt = pp_sm.tile([16, 128], BF16, tag="sm")
                    rows = (J - 1 - t) // NT_J + 1
                    nc.tensor.transpose(pt[:, :rows], q_all[:rows, t, :],
                                        ident[:rows, :rows])
                    dst = qt_rep[0:16, b, t::NT_J]
                    nc.scalar.copy(out=dst[:, :rows], in_=pt[:, :rows])
                for g in range(1, 4):
                    nc.sync.dma_start(out=qt_rep[32 * g:32 * g + 16, b, :],
                                      in_=qt_rep[0:16, b, :])

            # ---- phase B: attention + output per batch ----
            for b in range(NB):
                ps_o = pp_o.tile([128, 512], F32, tag="po")
                for it in range(NT_I):
                    rows = _rows_i(it)
                    ps_s = pp_s.tile([128, 2048], F32, tag="ps")
                    for g, (c0, w) in enumerate(CHUNKS):
                        nc.tensor.matmul(
                            out=ps_s[:rows, c0:c0 + w],
                            lhsT=kt_rep[32 * g:32 * g + 16, b,
                                        it * 128:it * 128 + rows],
                            rhs=qt_rep[32 * g:32 * g + 16, b, c0:c0 + w],
                            start=True, stop=True, tile_position=(32 * g, 0))
                    e_sb = ebuf.tile([128, 1568], BF16, tag="e")
                    zcol = small.tile([128, 1], F32, tag="z")
                    nc.scalar.activation(out=e_sb[:rows, :], in_=ps_s[:rows, 0:1568],
                                         func=AF.Exp, scale=SCALE,
                                         accum_out=zcol[:rows, :])
                    rcol = small.tile([128, 1], F32, tag="r")
                    nc.vector.reciprocal(out=rcol[:rows, :], in_=zcol[:rows, :])
                    kp = small.tile([128, 16], BF16, tag="kp")
                    nc.vector.tensor_scalar_mul(out=kp[:rows, :],
                                                in0=k_sb[:rows, b, it, :],
                                                scalar1=rcol[:rows, :])
                    for g, (c0, w) in enumerate(CHUNKS):
                        nc.tensor.matmul(
                            out=ps_o[32 * g:32 * g + 16, 0:w],
                            lhsT=kp[:rows, :], rhs=e_sb[:rows, c0:c0 + w],
                            start=(it == 0), stop=(it == NT_I - 1),
                            tile_position=(0, 32 * g), skip_group_check=True)

                # Out^T -> flat Out rows (p-major: partition p = flat 208p..)
                otf = perb.tile([16, 1568], BF16, tag="ot")
                for g, (c0, w) in enumerate(CHUNKS):
                    nc.vector.tensor_copy(out=otf[:, c0:c0 + w],
                                          in_=ps_o[32 * g:32 * g + 16, 0:w])
                fo_sb = perb.tile([128, NT_J, 16], BF16, tag="fos")
                nc.vector.memset(fo_sb, 0.0)
                ps_ot = pp_sm.tile([128, 208], BF16, tag="sm")
                for t in range(NT_J):
                    rows = (J - 1 - t) // NT_J + 1
                    nc.tensor.transpose(ps_ot[:rows, t * 16:(t + 1) * 16],
                                        otf[:, t::NT_J][:, :rows], bident[0:16, :])
                    nc.vector.tensor_copy(out=fo_sb[:rows, t, :],
                                          in_=ps_ot[:rows, t * 16:(t + 1) * 16])
                nc.sync.dma_start(
                    out=fo[b, :].rearrange("(p t e) -> p t e", p=128, e=16),
                    in_=fo_sb)

                # convout
                for blk in range(4):
                    x2p = small.tile([128, 9, 9], BF16, tag="x2p")
                    nc.vector.memset(x2p, 0.0)
                    nc.sync.dma_start(
                        out=x2p[:, 1:8, 1:8],
                        in_=fo[b, ds(blk * 6272, 6272)].rearrange(
                            "(p y x) -> p y x", p=128, y=7))
                    pc = pp_sm.tile([128, 49], F32, tag="sm")
                    for tap in range(9):
                        ky, kx = tap // 3, tap % 3
                        nc.tensor.matmul(
                            out=pc[:, :], lhsT=wqs[:, tap, blk, :],
                            rhs=x2p[:, ky:ky + 7, kx:kx + 7],
                            start=(tap == 0), stop=(tap == 8))
                    x3c = small.tile([128, 49], F32, tag="x3c")
                    nc.vector.tensor_copy(out=x3c, in_=pc)
                    nc.sync.dma_start(
                        out=f3[b, ds(blk * 6272, 6272)].rearrange("(p s) -> p s", p=128),
                        in_=x3c)

                # LayerNorm over 16-elem groups of flat f3
                nc.sync.dma_start(
                    out=f3[b, ds(J * 16, 1536)].rearrange("(p e) -> p e", p=96),
                    in_=zpad)
                y = perb.tile([128, NT_J, 16], F32, tag="y")
                nc.sync.dma_start(
                    out=y, in_=f3[b, :].rearrange("(p t e) -> p t e", p=128, e=16))
                sums = small.tile([128, NT_J], F32, tag="sums")
                nc.vector.tensor_reduce(out=sums, in_=y, axis=mybir.AxisListType.X,
                                        op=ALU.add)
                sq = perb.tile([128, NT_J, 16], F32, tag="sq")
                nc.vector.tensor_mul(out=sq, in0=y, in1=y)
                sqs = small.tile([128, NT_J], F32, tag="sqs")
                nc.vector.tensor_reduce(out=sqs, in_=sq, axis=mybir.AxisListType.X,
                                        op=ALU.add)
                mu = small.tile([128, NT_J], F32, tag="mu")
                nc.vector.tensor_scalar_mul(out=mu, in0=sums, scalar1=1.0 / 16)
                msq = small.tile([128, NT_J], F32, tag="msq")
                nc.vector.tensor_mul(out=msq, in0=mu, in1=mu)
                var = small.tile([128, NT_J], F32, tag="var")
                nc.vector.scalar_tensor_tensor(out=var, in0=sqs, scalar=1.0 / 16,
                                               in1=msq, op0=ALU.mult,
                                               op1=ALU.subtract)
                # rstd = 1/sqrt(var+eps): bit-trick init + 2 Newton iters (DVE)
                vpe = small.tile([128, NT_J], F32, tag="vpe")
                nc.vector.tensor_scalar_add(out=vpe, in0=var, scalar1=EPS)
                rstd = small.tile([128, NT_J], F32, tag="rstd")
                ri = rstd[:, :].bitcast(mybir.dt.int32)
                nc.vector.tensor_scalar(
                    out=ri, in0=vpe[:, :].bitcast(mybir.dt.int32), scalar1=1,
                    scalar2=None, op0=ALU.logical_shift_right)
                nc.vector.tensor_scalar(
                    out=ri, in0=ri, scalar1=-1, scalar2=0x5F3759DF,
                    op0=ALU.mult, op1=ALU.add)
                tnw = small.tile([128, NT_J], F32, tag="tnw")
                for _ in range(2):
                    nc.vector.tensor_mul(out=tnw, in0=rstd, in1=rstd)
                    nc.vector.tensor_mul(out=tnw, in0=tnw, in1=vpe)
                    nc.vector.tensor_scalar(
                        out=tnw, in0=tnw, scalar1=-0.5, scalar2=1.5,
                        op0=ALU.mult, op1=ALU.add)
                    nc.vector.tensor_mul(out=rstd, in0=rstd, in1=tnw)
                yn = perb.tile([128, NT_J, 16], F32, tag="yn")
                for jt in range(NT_J):
                    nc.vector.tensor_scalar(
                        out=yn[:, jt, :], in0=y[:, jt, :],
                        scalar1=mu[:, jt:jt + 1], scalar2=rstd[:, jt:jt + 1],
                        op0=ALU.subtract, op1=ALU.mult)
                nc.vector.tensor_mul(out=yn, in0=yn, in1=gam_b)
                nc.vector.tensor_add(out=yn, in0=yn, in1=bet_b)
                nc.sync.dma_start(
                    out=outy[b, :].rearrange("(p t e) -> p t e", p=128, e=16),
                    in_=yn)

    nc.compile()
    return nc


def _blockdiag(w):
    out = np.zeros((9, 4, 128, 128), np.float32)
    for blk in range(4):
        for g in range(8):
            grp = blk * 8 + g
            for ky in range(3):
                for kx in range(3):
                    out[ky * 3 + kx, blk, g * 16:(g + 1) * 16, g * 16:(g + 1) * 16] = \
                        w[grp * 16:(grp + 1) * 16, :, ky, kx].T
    return out


def kernel(current_pose, next_pose, current_w, next_w, E_proj, rel_embedd,
           ln_gamma, ln_beta, num_iter=None):
    global _PROG
    if _PROG is None:
        _PROG = _build()

    bf = ml_dtypes.bfloat16
    cp_raw = np.ascontiguousarray(
        np.asarray(current_pose, np.float32).transpose(0, 1, 4, 2, 3)
    ).reshape(B, C, H, H)
    cp_img = np.zeros((B, C, 16, 16), np.float32)
    cp_img[:, :, 1:15, 1:15] = cp_raw
    cp_img = cp_img.astype(bf)
    qp_raw = np.ascontiguousarray(
        np.asarray(next_pose, np.float32).transpose(0, 1, 4, 2, 3)
    ).reshape(B, C, HO, HO)
    qp_img = np.zeros((B, C, 9, 9), np.float32)
    qp_img[:, :, 1:8, 1:8] = qp_raw
    qp_img = qp_img.astype(bf)
    w1bd = _blockdiag(np.asarray(current_w, np.float32)).astype(bf)
    wqbd = _blockdiag(np.asarray(next_w, np.float32)).astype(bf)
    ep = np.asarray(E_proj, np.float32).astype(bf)
    rel = np.asarray(rel_embedd, np.float32)
    ident = np.eye(128, dtype=np.float32).astype(bf)
    bident = np.zeros((128, 16), np.float32)
    for p in range(128):
        if p % 32 < 16:
            bident[p, p % 32] = 1.0
    bident = bident.astype(bf)

    common = {
        "w1bd": w1bd, "wqbd": wqbd, "ep": ep,
        "rel_k": np.ascontiguousarray(rel.T).astype(np.float32),
        "rel_kt": rel.astype(bf),
        "gam": np.asarray(ln_gamma, np.float32),
        "bet": np.asarray(ln_beta, np.float32),
        "ident": ident, "bident": bident,
    }
    core_ids = list(range(8))
    in_maps = []
    for c in core_ids:
        sl = slice(c * NB, (c + 1) * NB)
        in_maps.append({**common, "cp_img": np.ascontiguousarray(cp_img[sl]),
                        "qp_img": np.ascontiguousarray(qp_img[sl])})

    res = run_bass_kernel_spmd(_PROG, in_maps, core_ids)
    out = np.empty((B, J * 16), np.float32)
    for c in core_ids:
        out[c * NB:(c + 1) * NB] = res.results[c]["outy"][:, :J * 16]
    return out.reshape(B, OUT_N, HO, HO, OUT_D)


if __name__ == "__main__":
    import reference as ref
    inputs = ref.setup_inputs()
    expected = np.asarray(ref.reference(**inputs))
    actual = kernel(**{k: np.asarray(v) if not np.isscalar(v) else v
                       for k, v in inputs.items()})
    err = np.abs(actual - expected)
    sc = np.abs(expected).max()
    print("absmax err:", err.max(), "scale:", sc, "rel:", err.max() / sc)


# revision 20
# speedup vs baseline: 1.0181x; 1.0181x over previous
"""Trainium2 Bass kernel for nn_BilinearLinformerCapsuleFC.

Strategy: data-parallel over batch (32 -> 4 per core x 8 cores).
Per core, per batch item:
  conv1/convq as block-diagonal grouped-conv matmuls (9 shifted taps,
  PSUM-accumulated), Linformer key projection per capsule, keys/queries
  transposed on the PE, then column-softmax attention computed in
  S^T [keys, queries] layout: row-group-packed K@Q^T matmuls -> one big
  ACT exp with accum_out row-sum (the softmax-over-queries normalizer)
  -> K' = K/Z -> col-group-packed K'^T @ E matmuls accumulating Out^T in
  a single PSUM bank. Output conv + LayerNorm (rsqrt via exp(-0.5 ln))
  with DRAM-roundtrip relayouts for the 16<->49 flat reinterpretations.
"""
import numpy as np
import ml_dtypes

import concourse.bass as bass
import concourse.mybir as mybir
import concourse.tile as tile
from concourse import bacc
from concourse.bass import ds
from concourse.bass_utils import run_bass_kernel_spmd

BF16 = mybir.dt.bfloat16
F32 = mybir.dt.float32
AF = mybir.ActivationFunctionType
ALU = mybir.AluOpType

B, IN_N, IN_D, H, OUT_N, OUT_D, HO, HID = 32, 32, 16, 14, 32, 16, 7, 64
C = IN_N * IN_D            # 512
NB = 4                     # batch items per core
NKEY = IN_N * HID + HO * HO  # 2097
J = OUT_N * HO * HO        # 1568
NT_I = 17                  # i tiles (16x128 + 49)
NT_J = 13                  # j tiles (12x128 + 32)
JPAD = NT_J * 2048         # 26624, padded flat len per batch
EPS = 1e-5
CHUNKS = [(0, 512), (512, 512), (1024, 512), (1536, 32)]  # j chunks
SCALE = IN_D ** -0.5       # 0.25

_PROG = None


def _rows_i(it):
    return 128 if it < 16 else NKEY - 16 * 128  # 49


def _rows_j(jt):
    return 128 if jt < 12 else J - 12 * 128  # 32


def _build():
    nc = bacc.Bacc("TRN2", target_bir_lowering=False, debug=False, num_devices=1)

    cp_img = nc.dram_tensor("cp_img", [NB, C, 16, 16], BF16, kind="ExternalInput")
    qp_img = nc.dram_tensor("qp_img", [NB, C, 9, 9], BF16, kind="ExternalInput")
    w1bd = nc.dram_tensor("w1bd", [9, 4, 128, 128], BF16, kind="ExternalInput")
    wqbd = nc.dram_tensor("wqbd", [9, 4, 128, 128], BF16, kind="ExternalInput")
    ep = nc.dram_tensor("ep", [IN_N, 196, HID], BF16, kind="ExternalInput")
    rel_k = nc.dram_tensor("rel_k", [49, 16], F32, kind="ExternalInput")
    rel_kt = nc.dram_tensor("rel_kt", [16, 49], BF16, kind="ExternalInput")
    gam_d = nc.dram_tensor("gam", [16], F32, kind="ExternalInput")
    bet_d = nc.dram_tensor("bet", [16], F32, kind="ExternalInput")
    ident_d = nc.dram_tensor("ident", [128, 128], BF16, kind="ExternalInput")
    bident_d = nc.dram_tensor("bident", [128, 16], BF16, kind="ExternalInput")

    fq = nc.dram_tensor("fq", [NB, JPAD], BF16)
    fo = nc.dram_tensor("fo", [NB, JPAD], BF16)
    f3 = nc.dram_tensor("f3", [NB, JPAD], F32)
    outy = nc.dram_tensor("outy", [NB, JPAD], F32, kind="ExternalOutput")

    def dmae(i):
        return nc.sync

    with tile.TileContext(nc) as tc:
        from contextlib import ExitStack
        with ExitStack() as ctx:
            consts = ctx.enter_context(tc.tile_pool(name="consts", bufs=1))
            big = ctx.enter_context(tc.tile_pool(name="big", bufs=1))
            perb = ctx.enter_context(tc.tile_pool(name="perb", bufs=2))
            ebuf = ctx.enter_context(tc.tile_pool(name="ebuf", bufs=2))
            small = ctx.enter_context(tc.tile_pool(name="small", bufs=3))
            pp_s = ctx.enter_context(tc.tile_pool(name="pp_s", bufs=1, space="PSUM"))
            pp_o = ctx.enter_context(tc.tile_pool(name="pp_o", bufs=2, space="PSUM"))
            pp_sm = ctx.enter_context(tc.tile_pool(name="pp_sm", bufs=2, space="PSUM"))

            # ---- constants ----
            ident = consts.tile([128, 128], BF16)
            nc.sync.dma_start(out=ident, in_=ident_d[:, :])
            bident = consts.tile([128, 16], BF16)
            nc.sync.dma_start(out=bident, in_=bident_d[:, :])
            w1s = consts.tile([128, 9, 4, 128], BF16)
            nc.sync.dma_start(out=w1s, in_=bass.AP(
                tensor=w1bd, offset=0,
                ap=[[128, 128], [4 * 128 * 128, 9], [128 * 128, 4], [1, 128]]))
            wqs = consts.tile([128, 9, 4, 128], BF16)
            nc.sync.dma_start(out=wqs, in_=bass.AP(
                tensor=wqbd, offset=0,
                ap=[[128, 128], [4 * 128 * 128, 9], [128 * 128, 4], [1, 128]]))
            eps_s = consts.tile([98, 2, IN_N, HID], BF16)
            for hf in range(2):
                nc.sync.dma_start(out=eps_s[:, hf, :, :], in_=bass.AP(
                    tensor=ep, offset=hf * 98 * HID,
                    ap=[[HID, 98], [196 * HID, IN_N], [1, HID]]))
            gam_b = consts.tile([128, NT_J, 16], F32)
            nc.sync.dma_start(out=gam_b, in_=bass.AP(
                tensor=gam_d, offset=0, ap=[[0, 128], [0, NT_J], [1, 16]]))
            bet_b = consts.tile([128, NT_J, 16], F32)
            nc.sync.dma_start(out=bet_b, in_=bass.AP(
                tensor=bet_d, offset=0, ap=[[0, 128], [0, NT_J], [1, 16]]))
            eps_t = consts.tile([128, 1], F32)
            nc.vector.memset(eps_t, EPS)
            zpad = consts.tile([96, 16], F32)
            nc.vector.memset(zpad, 0.0)

            # ---- phase A: inputs come pre-padded from host ----
            x0 = big.tile([128, NB, 4, 16, 16], BF16)
            x0q = big.tile([128, NB, 4, 9, 9], BF16)
            for b in range(NB):
                for blk in range(4):
                    nc.sync.dma_start(out=x0[:, b, blk, :, :],
                                      in_=cp_img[b, blk * 128:(blk + 1) * 128, :, :])
                    nc.sync.dma_start(out=x0q[:, b, blk, :, :],
                                      in_=qp_img[b, blk * 128:(blk + 1) * 128, :, :])

            # ---- conv1 (block-diag, tap-outer for weight reuse) ----
            x1 = big.tile([98, NB, 2, C], BF16)   # X1^T: [s_lo, b, s_half, c]
            for blk in range(4):
                for bp in range(2):
                    pc = pp_sm.tile([128, 2, 196], F32, tag="sm")
                    for tap in range(9):
                        ky, kx = tap // 3, tap % 3
                        nc.tensor.matmul(
                            out=pc[:, :, :], lhsT=w1s[:, tap, blk, :],
                            rhs=x0[:, bp * 2:bp * 2 + 2, blk,
                                   ky:ky + 14, kx:kx + 14],
                            start=(tap == 0), stop=(tap == 8))
                    for i in range(2):
                        b = bp * 2 + i
                        x1c = small.tile([128, 196], BF16, tag="x1c")
                        nc.vector.tensor_copy(out=x1c, in_=pc[:, i, :])
                        for hf in range(2):
                            pt = pp_sm.tile([98, 128], BF16, tag="sm")
                            nc.tensor.transpose(pt, x1c[:, hf * 98:(hf + 1) * 98],
                                                ident)
                            nc.vector.tensor_copy(
                                out=x1[:, b, hf, blk * 128:(blk + 1) * 128], in_=pt)

            # ---- k projection (per capsule), K assembly ----
            kt_rep = big.tile([128, NB, 2112], BF16)  # K^T replicas at part 0/32/64/96
            k_sb = big.tile([128, NB, NT_I, 16], F32)
            for b in range(NB):
                nc.sync.dma_start(out=k_sb[0:49, b, 16, :], in_=rel_k[:, :])
            for m in range(2):          # capsule halves (16 each)
                pk = pp_sm.tile([128, NB, 8, 16], F32, tag="sm")
                for q in range(16):
                    n2 = m * 16 + q
                    pslice = pk[64 * (q % 2):64 * (q % 2) + 64, :, q // 2, :]
                    tp = (0, 64) if (q % 2) else (0, 0)
                    for hf in range(2):
                        nc.tensor.matmul(
                            out=pslice, lhsT=eps_s[:, hf, n2, :],
                            rhs=x1[:, :, hf, n2::32],
                            start=(hf == 0), stop=(hf == 1),
                            tile_position=tp)
                for b in range(NB):
                    nc.vector.tensor_copy(out=k_sb[:, b, m * 8:(m + 1) * 8, :],
                                          in_=pk[:, b, :, :])

            # K -> bf16 -> K^T via PE transpose; replicate to row groups
            for b in range(NB):
                kbf = perb.tile([128, 16, 16], BF16, tag="kbf")
                nc.vector.tensor_copy(out=kbf, in_=k_sb[:, b, 0:16, :])
                for grp in range(4):
                    pt = pp_sm.tile([16, 512], BF16, tag="sm")
                    for u in range(4):
                        t = grp * 4 + u
                        nc.tensor.transpose(pt[:, u * 128:(u + 1) * 128],
                                            kbf[:, t, :], ident)
                    nc.scalar.copy(
                        out=kt_rep[0:16, b, grp * 512:(grp + 1) * 512], in_=pt)
                nc.sync.dma_start(out=kt_rep[0:16, b, 2048:2097], in_=rel_kt[:, :])
                for g in range(1, 4):
                    nc.sync.dma_start(out=kt_rep[32 * g:32 * g + 16, b, 0:2097],
                                      in_=kt_rep[0:16, b, 0:2097])

            # ---- convq -> fq roundtrip -> Q^T ----
            qt_rep = big.tile([128, NB, 1568], BF16)
            for blk in range(4):
                pc = pp_sm.tile([128, NB, 49], F32, tag="sm")
                for tap in range(9):
                    ky, kx = tap // 3, tap % 3
                    nc.tensor.matmul(
                        out=pc[:, :, :], lhsT=wqs[:, tap, blk, :],
                        rhs=x0q[:, :, blk, ky:ky + 7, kx:kx + 7],
                        start=(tap == 0), stop=(tap == 8))
                for b in range(NB):
                    xqc = small.tile([128, 49], BF16, tag="xqc")
                    nc.vector.tensor_copy(out=xqc, in_=pc[:, b, :])
                    nc.sync.dma_start(
                        out=fq[b, ds(blk * 6272, 6272)].rearrange("(p s) -> p s", p=128),
                        in_=xqc)
            for b in range(NB):
                # p-major reload: partition p holds flat [208p, 208p+208)
                q_all = perb.tile([128, NT_J, 16], BF16, tag="qall")
                nc.sync.dma_start(
                    out=q_all,
                    in_=fq[b, :].rearrange("(p t e) -> p t e", p=128, e=16))
                for t in range(NT_J):
                    # transpose j-rows {13p + t} -> QT cols strided by 13
                    pt = pp_sm.tile([16, 128], BF16, tag="sm")
                    rows = (J - 1 - t) // NT_J + 1
                    nc.tensor.transpose(pt[:, :rows], q_all[:rows, t, :],
                                        ident[:rows, :rows])
                    dst = qt_rep[0:16, b, t::NT_J]
                    nc.scalar.copy(out=dst[:, :rows], in_=pt[:, :rows])
                for g in range(1, 4):
                    nc.sync.dma_start(out=qt_rep[32 * g:32 * g + 16, b, :],
                                      in_=qt_rep[0:16, b, :])

            # ---- phase B: attention + output per batch ----
            for b in range(NB):
                ps_o = pp_o.tile([128, 512], F32, tag="po")
                for it in range(NT_I):
                    rows = _rows_i(it)
                    ps_s = pp_s.tile([128, 2048], F32, tag="ps")
                    for g, (c0, w) in enumerate(CHUNKS):
                        nc.tensor.matmul(
                            out=ps_s[:rows, c0:c0 + w],
                            lhsT=kt_rep[32 * g:32 * g + 16, b,
                                        it * 128:it * 128 + rows],
                            rhs=qt_rep[32 * g:32 * g + 16, b, c0:c0 + w],
                            start=True, stop=True, tile_position=(32 * g, 0))
                    e_sb = ebuf.tile([128, 1568], BF16, tag="e")
                    zcol = small.tile([128, 1], F32, tag="z")
                    nc.scalar.activation(out=e_sb[:rows, :], in_=ps_s[:rows, 0:1568],
                                         func=AF.Exp, scale=SCALE,
                                         accum_out=zcol[:rows, :])
                    rcol = small.tile([128, 1], F32, tag="r")
                    nc.vector.reciprocal(out=rcol[:rows, :], in_=zcol[:rows, :])
                    kp = small.tile([128, 16], BF16, tag="kp")
                    nc.vector.tensor_scalar_mul(out=kp[:rows, :],
                                                in0=k_sb[:rows, b, it, :],
                                                scalar1=rcol[:rows, :])
                    for g, (c0, w) in enumerate(CHUNKS):
                        nc.tensor.matmul(
                            out=ps_o[32 * g:32 * g + 16, 0:w],
                            lhsT=kp[:rows, :], rhs=e_sb[:rows, c0:c0 + w],
                            start=(it == 0), stop=(it == NT_I - 1),
                            tile_position=(0, 32 * g), skip_group_check=True)

                # Out^T -> flat Out rows (p-major: partition p = flat 208p..)
                otf = perb.tile([16, 1568], BF16, tag="ot")
                for g, (c0, w) in enumerate(CHUNKS):
                    nc.vector.tensor_copy(out=otf[:, c0:c0 + w],
                                          in_=ps_o[32 * g:32 * g + 16, 0:w])
                fo_sb = perb.tile([128, NT_J, 16], BF16, tag="fos")
                nc.vector.memset(fo_sb, 0.0)
                ps_ot = pp_sm.tile([128, 208], BF16, tag="sm")
                for t in range(NT_J):
                    rows = (J - 1 - t) // NT_J + 1
                    nc.tensor.transpose(ps_ot[:rows, t * 16:(t + 1) * 16],
                                        otf[:, t::NT_J][:, :rows], bident[0:16, :])
                    nc.vector.tensor_copy(out=fo_sb[:rows, t, :],
                                          in_=ps_ot[:rows, t * 16:(t + 1) * 16])
                nc.sync.dma_start(
                    out=fo[b, :].rearrange("(p t e) -> p t e", p=128, e=16),
                    in_=fo_sb)

                # convout
                for blk in range(4):
                    xt = small.tile([128, 49], BF16, tag="xt")
                    nc.sync.dma_start(
                        out=xt,
                        in_=fo[b, ds(blk * 6272, 6272)].rearrange(
                            "(p s) -> p s", p=128))
                    x2p = small.tile([128, 9, 9], BF16, tag="x2p")
                    nc.vector.memset(x2p, 0.0)
                    nc.vector.tensor_copy(
                        out=x2p[:, 1:8, 1:8],
                        in_=xt.rearrange("p (y x) -> p y x", y=7))
                    pc = pp_sm.tile([128, 49], F32, tag="sm")
                    for tap in range(9):
                        ky, kx = tap // 3, tap % 3
                        nc.tensor.matmul(
                            out=pc[:, :], lhsT=wqs[:, tap, blk, :],
                            rhs=x2p[:, ky:ky + 7, kx:kx + 7],
                            start=(tap == 0), stop=(tap == 8))
                    x3c = small.tile([128, 49], F32, tag="x3c")
                    nc.vector.tensor_copy(out=x3c, in_=pc)
                    nc.sync.dma_start(
                        out=f3[b, ds(blk * 6272, 6272)].rearrange("(p s) -> p s", p=128),
                        in_=x3c)

                # LayerNorm over 16-elem groups of flat f3
                nc.sync.dma_start(
                    out=f3[b, ds(J * 16, 1536)].rearrange("(p e) -> p e", p=96),
                    in_=zpad)
                y = perb.tile([128, NT_J, 16], F32, tag="y")
                nc.sync.dma_start(
                    out=y, in_=f3[b, :].rearrange("(p t e) -> p t e", p=128, e=16))
                sums = small.tile([128, NT_J], F32, tag="sums")
                nc.vector.tensor_reduce(out=sums, in_=y, axis=mybir.AxisListType.X,
                                        op=ALU.add)
                sq = perb.tile([128, NT_J, 16], F32, tag="sq")
                nc.vector.tensor_mul(out=sq, in0=y, in1=y)
                sqs = small.tile([128, NT_J], F32, tag="sqs")
                nc.vector.tensor_reduce(out=sqs, in_=sq, axis=mybir.AxisListType.X,
                                        op=ALU.add)
                mu = small.tile([128, NT_J], F32, tag="mu")
                nc.vector.tensor_scalar_mul(out=mu, in0=sums, scalar1=1.0 / 16)
                msq = small.tile([128, NT_J], F32, tag="msq")
                nc.vector.tensor_mul(out=msq, in0=mu, in1=mu)
                var = small.tile([128, NT_J], F32, tag="var")
                nc.vector.scalar_tensor_tensor(out=var, in0=sqs, scalar=1.0 / 16,
                                               in1=msq, op0=ALU.mult,
                                               op1=ALU.subtract)
                # rstd = 1/sqrt(var+eps): bit-trick init + 2 Newton iters (DVE)
                vpe = small.tile([128, NT_J], F32, tag="vpe")
                nc.vector.tensor_scalar_add(out=vpe, in0=var, scalar1=EPS)
                rstd = small.tile([128, NT_J], F32, tag="rstd")
                ri = rstd[:, :].bitcast(mybir.dt.int32)
                nc.vector.tensor_scalar(
                    out=ri, in0=vpe[:, :].bitcast(mybir.dt.int32), scalar1=1,
                    scalar2=None, op0=ALU.logical_shift_right)
                nc.vector.tensor_scalar(
                    out=ri, in0=ri, scalar1=-1, scalar2=0x5F3759DF,
                    op0=ALU.mult, op1=ALU.add)
                tnw = small.tile([128, NT_J], F32, tag="tnw")
                for _ in range(2):
                    nc.vector.tensor_mul(out=tnw, in0=rstd, in1=rstd)
                    nc.vector.tensor_mul(out=tnw, in0=tnw, in1=vpe)
                    nc.vector.tensor_scalar(
                        out=tnw, in0=tnw, scalar1=-0.5, scalar2=1.5,
                        op0=ALU.mult, op1=ALU.add)
                    nc.vector.tensor_mul(out=rstd, in0=rstd, in1=tnw)
                yn = perb.tile([128, NT_J, 16], F32, tag="yn")
                for jt in range(NT_J):
                    nc.vector.tensor_scalar(
                        out=yn[:, jt, :], in0=y[:, jt, :],
                        scalar1=mu[:, jt:jt + 1], scalar2=rstd[:, jt:jt + 1],
                        op0=ALU.subtract, op1=ALU.mult)
                nc.vector.tensor_mul(out=yn, in0=yn, in1=gam_b)
                nc.vector.tensor_add(out=yn, in0=yn, in1=bet_b)
                nc.sync.dma_start(
                    out=outy[b, :].rearrange("(p t e) -> p t e", p=128, e=16),
                    in_=yn)

    nc.compile()
    return nc


def _blockdiag(w):
    out = np.zeros((9, 4, 128, 128), np.float32)
    for blk in range(4):
        for g in range(8):
            grp = blk * 8 + g
            for ky in range(3):
                for kx in range(3):
                    out[ky * 3 + kx, blk, g * 16:(g + 1) * 16, g * 16:(g + 1) * 16] = \
                        w[grp * 16:(grp + 1) * 16, :, ky, kx].T
    return out


def kernel(current_pose, next_pose, current_w, next_w, E_proj, rel_embedd,
           ln_gamma, ln_beta, num_iter=None):
    global _PROG
    if _PROG is None:
        _PROG = _build()

    bf = ml_dtypes.bfloat16
    cp_raw = np.ascontiguousarray(
        np.asarray(current_pose, np.float32).transpose(0, 1, 4, 2, 3)
    ).reshape(B, C, H, H)
    cp_img = np.zeros((B, C, 16, 16), np.float32)
    cp_img[:, :, 1:15, 1:15] = cp_raw
    cp_img = cp_img.astype(bf)
    qp_raw = np.ascontiguousarray(
        np.asarray(next_pose, np.float32).transpose(0, 1, 4, 2, 3)
    ).reshape(B, C, HO, HO)
    qp_img = np.zeros((B, C, 9, 9), np.float32)
    qp_img[:, :, 1:8, 1:8] = qp_raw
    qp_img = qp_img.astype(bf)
    w1bd = _blockdiag(np.asarray(current_w, np.float32)).astype(bf)
    wqbd = _blockdiag(np.asarray(next_w, np.float32)).astype(bf)
    ep = np.asarray(E_proj, np.float32).astype(bf)
    rel = np.asarray(rel_embedd, np.float32)
    ident = np.eye(128, dtype=np.float32).astype(bf)
    bident = np.zeros((128, 16), np.float32)
    for p in range(128):
        if p % 32 < 16:
            bident[p, p % 32] = 1.0
    bident = bident.astype(bf)

    common = {
        "w1bd": w1bd, "wqbd": wqbd, "ep": ep,
        "rel_k": np.ascontiguousarray(rel.T).astype(np.float32),
        "rel_kt": rel.astype(bf),
        "gam": np.asarray(ln_gamma, np.float32),
        "bet": np.asarray(ln_beta, np.float32),
        "ident": ident, "bident": bident,
    }
    core_ids = list(range(8))
    in_maps = []
    for c in core_ids:
        sl = slice(c * NB, (c + 1) * NB)
        in_maps.append({**common, "cp_img": np.ascontiguousarray(cp_img[sl]),
                        "qp_img": np.ascontiguousarray(qp_img[sl])})

    res = run_bass_kernel_spmd(_PROG, in_maps, core_ids)
    out = np.empty((B, J * 16), np.float32)
    for c in core_ids:
        out[c * NB:(c + 1) * NB] = res.results[c]["outy"][:, :J * 16]
    return out.reshape(B, OUT_N, HO, HO, OUT_D)


if __name__ == "__main__":
    import reference as ref
    inputs = ref.setup_inputs()
    expected = np.asarray(ref.reference(**inputs))
    actual = kernel(**{k: np.asarray(v) if not np.isscalar(v) else v
                       for k, v in inputs.items()})
    err = np.abs(actual - expected)
    sc = np.abs(expected).max()
    print("absmax err:", err.max(), "scale:", sc, "rel:", err.max() / sc)
